# revision 17
# baseline (speedup 1.0000x reference)
"""GCN edge-aggregation kernel for 8 Trainium2 NeuronCores.

Math (see nn_GCNEdge): h = relu((segment_sum(edge_data, dst) / max(count,1)) @ W.T + b)

Strategy
--------
Host-side (sharding/layout only — all arithmetic happens on device):
  * Nodes are split contiguously across the 8 cores (12544 = 98 blocks of 128
    nodes per core; 8*12544 = 100352 >= 100000).
  * Each edge is routed to the core/block owning its destination node (CSR-style
    destination binning).  Within a block, edges occupy sequential slots; each
    block is padded to K_CHUNKS*128 slots so the device program is data-independent.
  * Edge features are shipped as a bf16 hi/lo pair (hi = bf16(x),
    lo = bf16(x - hi)) so the on-device f32-accumulated matmuls reconstruct
    ~fp32 precision while streaming at bf16 rates.  A constant-1 column rides
    along for the degree counts.

Device-side (per core, per 128-node block):
  * One-hot matrix of local node ids (DVE is_equal against an iota row),
  * PE matmul-accumulate onehot.T @ [x_hi | 1 | x_lo | 0] into PSUM -> per-node
    feature sums (hi+lo parts) and counts,
  * mean = sums * reciprocal(max(count, 1)),
  * PE transpose, then out = relu(W @ agg.T + b) via a second matmul with the
    (pre-transposed) weight as the stationary operand; output stays transposed
    [out_feat, node] and is un-transposed on the host.

No collectives are needed: output shards are disjoint.
"""

import numpy as np
import ml_dtypes

BF16 = ml_dtypes.bfloat16

N_NODES = 100000
N_EDGES = 1600000
F = 128
N_CORES = 8
BLK = 128                       # nodes per block
BLOCKS_PER_CORE = 98
TOTAL_BLOCKS = N_CORES * BLOCKS_PER_CORE        # 784
NODES_PER_CORE = BLOCKS_PER_CORE * BLK          # 12544
K_CHUNKS = 18                   # 128-edge chunks per block (capacity 2304 edges)

_module_cache = {}


def _build_module(K):
    import concourse.mybir as mybir
    import concourse.tile as tile
    from concourse import bacc

    f32 = mybir.dt.float32
    bf16 = mybir.dt.bfloat16
    RB = K * 128                 # edge slots per block
    SLOTS = BLOCKS_PER_CORE * RB

    nc = bacc.Bacc("TRN2", target_bir_lowering=False, debug=False)
    # xe rows are (block, partition); each row is that partition's K chunks of
    # 258 bf16 values laid contiguously -> 9KB-contiguous DMA descriptors.
    xe = nc.dram_tensor("xe", [BLOCKS_PER_CORE * 128, K * 258], bf16, kind="ExternalInput")
    lid = nc.dram_tensor("lid", [128, BLOCKS_PER_CORE * K], bf16, kind="ExternalInput")
    wt = nc.dram_tensor("wt", [128, 128], f32, kind="ExternalInput")
    bias = nc.dram_tensor("bias", [128, 1], f32, kind="ExternalInput")
    ident = nc.dram_tensor("ident", [128, 128], f32, kind="ExternalInput")
    # iota value pattern tiled K times: iotar[p, c*128 + f] = f
    iotar = nc.dram_tensor("iotar", [128, K * 128], bf16, kind="ExternalInput")
    out = nc.dram_tensor("out", [128, BLOCKS_PER_CORE * 128], f32, kind="ExternalOutput")

    xe_ap = xe.ap()
    out_ap = out.ap()

    with tile.TileContext(nc) as tc:
        with (
            tc.tile_pool(name="const", bufs=1) as cpool,
            tc.tile_pool(name="xp", bufs=6) as xpool,
            tc.tile_pool(name="ohp", bufs=4) as ohpool,
            tc.tile_pool(name="ep", bufs=3) as epool,
            tc.tile_pool(name="psS", bufs=4, space="PSUM") as psS,
            tc.tile_pool(name="psT", bufs=2, space="PSUM") as psT,
            tc.tile_pool(name="psO", bufs=2, space="PSUM") as psO,
        ):
            wt_t = cpool.tile([128, 128], f32)
            nc.sync.dma_start(wt_t[:], wt.ap()[:])
            bias_t = cpool.tile([128, 1], f32)
            nc.sync.dma_start(bias_t[:], bias.ap()[:])
            id_t = cpool.tile([128, 128], f32)
            nc.sync.dma_start(id_t[:], ident.ap()[:])
            iotar_t = cpool.tile([128, K * 128], bf16)
            nc.sync.dma_start(iotar_t[:], iotar.ap()[:])
            lid_t = cpool.tile([128, BLOCKS_PER_CORE * K], bf16)
            nc.sync.dma_start(lid_t[:], lid.ap()[:])

            group_pT = {}

            def emit_block(b, xt, oh):
                ps = psS.tile([128, 258], f32, name=f"ps{b}", tag="ps")
                for c in range(K):
                    nc.tensor.matmul(
                        ps[:],
                        lhsT=oh[:, c * 128:(c + 1) * 128],
                        rhs=xt[:, c * 258:(c + 1) * 258],
                        start=(c == 0),
                        stop=(c == K - 1),
                    )
                # counts live in ps[:,128] (the lo-side count column is all
                # zeros by construction), so no hi+lo add is needed for them.
                cm = epool.tile([128, 1], f32, name=f"cm{b}", tag="cm")
                nc.vector.tensor_scalar_max(cm[:], ps[:, 128:129], 1.0)
                rec = epool.tile([128, 1], f32, name=f"rec{b}", tag="rec")
                nc.vector.reciprocal(rec[:], cm[:])
                # agg = (S_hi + S_lo) / max(count,1), folded as
                # t1 = S_hi*rec (ACT), agg = S_lo*rec + t1 (one fused DVE op)
                t1 = epool.tile([128, 128], f32, name=f"t1{b}", tag="t1")
                nc.scalar.activation(
                    t1[:], ps[:, 0:128],
                    mybir.ActivationFunctionType.Copy, scale=rec[:, 0:1],
                )
                agg = epool.tile([128, 128], f32, name=f"agg{b}", tag="agg")
                nc.vector.scalar_tensor_tensor(
                    out=agg[:],
                    in0=ps[:, 129:257],
                    scalar=rec[:, 0:1],
                    in1=t1[:],
                    op0=mybir.AluOpType.mult,
                    op1=mybir.AluOpType.add,
                )
                j = b % 4
                if j == 0:
                    group_pT["t"] = psT.tile([128, 512], f32, name=f"pT{b}", tag="pT")
                pT = group_pT["t"]
                nc.tensor.transpose(pT[:, j * 128:(j + 1) * 128], agg[:], id_t[:])
                if j == 3 or b == BLOCKS_PER_CORE - 1:
                    g0 = (b // 4) * 4
                    gw = (b + 1 - g0) * 128
                    aggT = epool.tile([128, 512], f32, name=f"aggT{b}", tag="aggT", bufs=2)
                    nc.scalar.copy(aggT[:, 0:gw], pT[:, 0:gw])
                    pO = psO.tile([128, 512], f32, name=f"pO{b}", tag="pO")
                    nc.tensor.matmul(
                        pO[:, 0:gw], lhsT=wt_t[:], rhs=aggT[:, 0:gw],
                        start=True, stop=True,
                    )
                    ot = epool.tile([128, 512], f32, name=f"ot{b}", tag="ot", bufs=2)
                    nc.scalar.activation(
                        ot[:, 0:gw], pO[:, 0:gw],
                        mybir.ActivationFunctionType.Relu,
                        bias=bias_t[:, 0:1], scale=1.0,
                    )
                    nc.sync.dma_start(out_ap[:, g0 * 128:(b + 1) * 128], ot[:, 0:gw])

            # Software-pipelined emission: engine queues are strict in-order,
            # so block b+1's (long) one-hot build must be emitted BEFORE block
            # b's PSUM-gated epilogue ops or it stalls behind them on DVE and
            # starves the PE at every block boundary.
            pending = {}
            for b in range(BLOCKS_PER_CORE):
                xt = xpool.tile([128, K * 258], bf16, name=f"xt{b}", tag="xt")
                nc.sync.dma_start(xt[:], xe_ap[b * 128:(b + 1) * 128, :])
                oh = ohpool.tile([128, K * 128], bf16, name=f"oh{b}", tag="oh")
                nc.vector.tensor_tensor(
                    out=oh[:].rearrange("p (c f) -> p c f", c=K),
                    in0=iotar_t[:].rearrange("p (c f) -> p c f", c=K),
                    in1=lid_t[:, b * K:(b + 1) * K].to_broadcast([128, K, 128]),
                    op=mybir.AluOpType.is_equal,
                )
                pending[b] = (xt, oh)
                if b >= 1:
                    emit_block(b - 1, *pending.pop(b - 1))
            emit_block(BLOCKS_PER_CORE - 1, *pending.pop(BLOCKS_PER_CORE - 1))

    nc.compile()
    return nc


def _get_module(K):
    if K not in _module_cache:
        _module_cache[K] = _build_module(K)
    return _module_cache[K]


def prepare_inputs(edge_data, dst, W, b):
    """Host-side sharding: route each edge to the core/block owning dst."""
    edge_data = np.asarray(edge_data, dtype=np.float32)
    dst = np.asarray(dst)
    W = np.asarray(W, dtype=np.float32)
    b = np.asarray(b, dtype=np.float32)
    E = dst.shape[0]

    blk = (dst.astype(np.int64)) >> 7                 # destination block id
    cnt = np.bincount(blk, minlength=TOTAL_BLOCKS)
    K = max(K_CHUNKS, int(np.ceil(cnt.max() / 128)))
    RB = K * 128
    TOT = TOTAL_BLOCKS * RB

    starts = np.zeros(TOTAL_BLOCKS, np.int64)
    np.cumsum(cnt[:-1], out=starts[1:])
    order = np.argsort(blk, kind="stable")
    rank = np.empty(E, np.int64)
    rank[order] = np.arange(E, dtype=np.int64) - np.repeat(starts, cnt)
    slot = blk * RB + rank

    X = np.zeros((TOT, 258), BF16)
    xh = edge_data.astype(BF16)
    X[slot, 0:128] = xh
    X[slot, 128] = BF16(1.0)
    X[slot, 129:257] = (edge_data - xh.astype(np.float32)).astype(BF16)
    # [block, chunk, partition, feat] -> [block, partition, chunk*feat] so each
    # SBUF partition's data is one long contiguous HBM run (big DMA descriptors).
    X = np.ascontiguousarray(
        X.reshape(TOTAL_BLOCKS, K, 128, 258).transpose(0, 2, 1, 3)
    ).reshape(N_CORES, BLOCKS_PER_CORE * 128, K * 258)

    lid_f = np.full(TOT, -1.0, np.float32)
    lid_f[slot] = (dst & 127).astype(np.float32)
    lid_all = (
        lid_f.reshape(N_CORES, BLOCKS_PER_CORE, K, 128)
        .transpose(0, 3, 1, 2)
        .reshape(N_CORES, 128, BLOCKS_PER_CORE * K)
        .astype(BF16)
    )
    wt = np.ascontiguousarray(W.T)
    bias = np.ascontiguousarray(b.reshape(128, 1))
    ident = np.eye(128, dtype=np.float32)
    iotar = np.ascontiguousarray(
        np.broadcast_to(
            np.arange(128, dtype=np.float32), (128, K, 128)
        ).reshape(128, K * 128)
    ).astype(BF16)

    in_maps = [
        {
            "xe": np.ascontiguousarray(X[c]),
            "lid": np.ascontiguousarray(lid_all[c]),
            "wt": wt,
            "bias": bias,
            "ident": ident,
            "iotar": iotar,
        }
        for c in range(N_CORES)
    ]
    return K, in_maps


def run(edge_data, dst, W, b, trace=False, tmpdir=None):
    from concourse.bass_utils import run_bass_kernel_spmd

    K, in_maps = prepare_inputs(edge_data, dst, W, b)
    nc = _get_module(K)
    res = run_bass_kernel_spmd(
        nc, in_maps, core_ids=list(range(N_CORES)), trace=trace, tmpdir=tmpdir,
    )
    outs = [res.results[c]["out"].T for c in range(N_CORES)]   # [12544, 128] each
    full = np.concatenate(outs, axis=0)[:N_NODES]
    return np.ascontiguousarray(full, dtype=np.float32), res


def kernel(edge_data, dst, W, b):
    out, _ = run(edge_data, dst, W, b, trace=False)
    return out


# revision 18
# speedup vs baseline: 1.0407x; 1.0407x over previous
"""GCN edge-aggregation kernel for 8 Trainium2 NeuronCores.

Math (see nn_GCNEdge): h = relu((segment_sum(edge_data, dst) / max(count,1)) @ W.T + b)

Strategy
--------
Host-side (sharding/layout only — all arithmetic happens on device):
  * Nodes are split contiguously across the 8 cores (12544 = 98 blocks of 128
    nodes per core; 8*12544 = 100352 >= 100000).
  * Each edge is routed to the core/block owning its destination node (CSR-style
    destination binning).  Within a block, edges occupy sequential slots; each
    block is padded to K_CHUNKS*128 slots so the device program is data-independent.
  * Edge features are shipped as a bf16 hi/lo pair (hi = bf16(x),
    lo = bf16(x - hi)) so the on-device f32-accumulated matmuls reconstruct
    ~fp32 precision while streaming at bf16 rates.  A constant-1 column rides
    along for the degree counts.

Device-side (per core, per 128-node block):
  * One-hot matrix of local node ids (DVE is_equal against an iota row),
  * PE matmul-accumulate onehot.T @ [x_hi | 1 | x_lo | 0] into PSUM -> per-node
    feature sums (hi+lo parts) and counts,
  * mean = sums * reciprocal(max(count, 1)),
  * PE transpose, then out = relu(W @ agg.T + b) via a second matmul with the
    (pre-transposed) weight as the stationary operand; output stays transposed
    [out_feat, node] and is un-transposed on the host.

No collectives are needed: output shards are disjoint.
"""

import numpy as np
import ml_dtypes

BF16 = ml_dtypes.bfloat16

N_NODES = 100000
N_EDGES = 1600000
F = 128
N_CORES = 8
BLK = 128                       # nodes per block
BLOCKS_PER_CORE = 98
TOTAL_BLOCKS = N_CORES * BLOCKS_PER_CORE        # 784
NODES_PER_CORE = BLOCKS_PER_CORE * BLK          # 12544
K_CHUNKS = 18                   # 128-edge chunks per block (capacity 2304 edges)

_module_cache = {}


def _build_module(K):
    import concourse.mybir as mybir
    import concourse.tile as tile
    from concourse import bacc

    f32 = mybir.dt.float32
    bf16 = mybir.dt.bfloat16
    RB = K * 128                 # edge slots per block
    SLOTS = BLOCKS_PER_CORE * RB

    nc = bacc.Bacc("TRN2", target_bir_lowering=False, debug=False)
    # xe rows are (block, partition); each row is that partition's K chunks of
    # 258 bf16 values laid contiguously -> 9KB-contiguous DMA descriptors.
    xe = nc.dram_tensor("xe", [BLOCKS_PER_CORE * 128, K * 258], bf16, kind="ExternalInput")
    lid = nc.dram_tensor("lid", [128, BLOCKS_PER_CORE * K], bf16, kind="ExternalInput")
    wt = nc.dram_tensor("wt", [128, 128], f32, kind="ExternalInput")
    bias = nc.dram_tensor("bias", [128, 1], f32, kind="ExternalInput")
    ident = nc.dram_tensor("ident", [128, 128], f32, kind="ExternalInput")
    # iota value pattern tiled K times: iotar[p, c*128 + f] = f
    iotar = nc.dram_tensor("iotar", [128, K * 128], bf16, kind="ExternalInput")
    out = nc.dram_tensor("out", [128, BLOCKS_PER_CORE * 128], f32, kind="ExternalOutput")

    xe_ap = xe.ap()
    out_ap = out.ap()

    with tile.TileContext(nc) as tc:
        with (
            tc.tile_pool(name="const", bufs=1) as cpool,
            tc.tile_pool(name="xp", bufs=6) as xpool,
            tc.tile_pool(name="ohp", bufs=8) as ohpool,
            tc.tile_pool(name="ep", bufs=3) as epool,
            tc.tile_pool(name="psS", bufs=4, space="PSUM") as psS,
            tc.tile_pool(name="psT", bufs=2, space="PSUM") as psT,
            tc.tile_pool(name="psO", bufs=2, space="PSUM") as psO,
        ):
            wt_t = cpool.tile([128, 128], f32)
            nc.sync.dma_start(wt_t[:], wt.ap()[:])
            bias_t = cpool.tile([128, 1], f32)
            nc.sync.dma_start(bias_t[:], bias.ap()[:])
            id_t = cpool.tile([128, 128], f32)
            nc.sync.dma_start(id_t[:], ident.ap()[:])
            iotar_t = cpool.tile([128, K * 128], bf16)
            nc.sync.dma_start(iotar_t[:], iotar.ap()[:])
            lid_t = cpool.tile([128, BLOCKS_PER_CORE * K], bf16)
            nc.sync.dma_start(lid_t[:], lid.ap()[:])

            group_pT = {}

            def emit_block(b, xt, oh):
                ps = psS.tile([128, 258], f32, name=f"ps{b}", tag="ps")
                for c in range(K):
                    nc.tensor.matmul(
                        ps[:],
                        lhsT=oh[:, c * 128:(c + 1) * 128],
                        rhs=xt[:, c * 258:(c + 1) * 258],
                        start=(c == 0),
                        stop=(c == K - 1),
                    )
                # counts live in ps[:,128] (the lo-side count column is all
                # zeros by construction), so no hi+lo add is needed for them.
                cm = epool.tile([128, 1], f32, name=f"cm{b}", tag="cm")
                nc.vector.tensor_scalar_max(cm[:], ps[:, 128:129], 1.0)
                rec = epool.tile([128, 1], f32, name=f"rec{b}", tag="rec")
                nc.vector.reciprocal(rec[:], cm[:])
                # agg = (S_hi + S_lo) / max(count,1), folded as
                # t1 = S_hi*rec (ACT), agg = S_lo*rec + t1 (one fused DVE op)
                t1 = epool.tile([128, 128], f32, name=f"t1{b}", tag="t1")
                nc.scalar.activation(
                    t1[:], ps[:, 0:128],
                    mybir.ActivationFunctionType.Copy, scale=rec[:, 0:1],
                )
                agg = epool.tile([128, 128], f32, name=f"agg{b}", tag="agg")
                nc.vector.scalar_tensor_tensor(
                    out=agg[:],
                    in0=ps[:, 129:257],
                    scalar=rec[:, 0:1],
                    in1=t1[:],
                    op0=mybir.AluOpType.mult,
                    op1=mybir.AluOpType.add,
                )
                j = b % 4
                if j == 0:
                    group_pT["t"] = psT.tile([128, 512], f32, name=f"pT{b}", tag="pT")
                pT = group_pT["t"]
                nc.tensor.transpose(pT[:, j * 128:(j + 1) * 128], agg[:], id_t[:])
                if j == 3 or b == BLOCKS_PER_CORE - 1:
                    g0 = (b // 4) * 4
                    gw = (b + 1 - g0) * 128
                    aggT = epool.tile([128, 512], f32, name=f"aggT{b}", tag="aggT", bufs=2)
                    nc.scalar.copy(aggT[:, 0:gw], pT[:, 0:gw])
                    pO = psO.tile([128, 512], f32, name=f"pO{b}", tag="pO")
                    nc.tensor.matmul(
                        pO[:, 0:gw], lhsT=wt_t[:], rhs=aggT[:, 0:gw],
                        start=True, stop=True,
                    )
                    ot = epool.tile([128, 512], f32, name=f"ot{b}", tag="ot", bufs=2)
                    nc.scalar.activation(
                        ot[:, 0:gw], pO[:, 0:gw],
                        mybir.ActivationFunctionType.Relu,
                        bias=bias_t[:, 0:1], scale=1.0,
                    )
                    nc.sync.dma_start(out_ap[:, g0 * 128:(b + 1) * 128], ot[:, 0:gw])

            # Software-pipelined emission: engine queues are strict in-order,
            # so block b+1's (long) one-hot build must be emitted BEFORE block
            # b's PSUM-gated epilogue ops or it stalls behind them on DVE and
            # starves the PE at every block boundary.
            pending = {}
            for b in range(BLOCKS_PER_CORE):
                xt = xpool.tile([128, K * 258], bf16, name=f"xt{b}", tag="xt")
                nc.sync.dma_start(xt[:], xe_ap[b * 128:(b + 1) * 128, :])
                oh = ohpool.tile([128, K * 128], bf16, name=f"oh{b}", tag="oh")
                with tc.high_priority(offset=100):
                    nc.vector.tensor_tensor(
                        out=oh[:].rearrange("p (c f) -> p c f", c=K),
                        in0=iotar_t[:].rearrange("p (c f) -> p c f", c=K),
                        in1=lid_t[:, b * K:(b + 1) * K].to_broadcast([128, K, 128]),
                        op=mybir.AluOpType.is_equal,
                    )
                pending[b] = (xt, oh)
                if b >= 1:
                    emit_block(b - 1, *pending.pop(b - 1))
            emit_block(BLOCKS_PER_CORE - 1, *pending.pop(BLOCKS_PER_CORE - 1))

    nc.compile()
    return nc


def _get_module(K):
    if K not in _module_cache:
        _module_cache[K] = _build_module(K)
    return _module_cache[K]


def prepare_inputs(edge_data, dst, W, b):
    """Host-side sharding: route each edge to the core/block owning dst."""
    edge_data = np.asarray(edge_data, dtype=np.float32)
    dst = np.asarray(dst)
    W = np.asarray(W, dtype=np.float32)
    b = np.asarray(b, dtype=np.float32)
    E = dst.shape[0]

    blk = (dst.astype(np.int64)) >> 7                 # destination block id
    cnt = np.bincount(blk, minlength=TOTAL_BLOCKS)
    K = max(K_CHUNKS, int(np.ceil(cnt.max() / 128)))
    RB = K * 128
    TOT = TOTAL_BLOCKS * RB

    starts = np.zeros(TOTAL_BLOCKS, np.int64)
    np.cumsum(cnt[:-1], out=starts[1:])
    order = np.argsort(blk, kind="stable")
    rank = np.empty(E, np.int64)
    rank[order] = np.arange(E, dtype=np.int64) - np.repeat(starts, cnt)
    slot = blk * RB + rank

    X = np.zeros((TOT, 258), BF16)
    xh = edge_data.astype(BF16)
    X[slot, 0:128] = xh
    X[slot, 128] = BF16(1.0)
    X[slot, 129:257] = (edge_data - xh.astype(np.float32)).astype(BF16)
    # [block, chunk, partition, feat] -> [block, partition, chunk*feat] so each
    # SBUF partition's data is one long contiguous HBM run (big DMA descriptors).
    X = np.ascontiguousarray(
        X.reshape(TOTAL_BLOCKS, K, 128, 258).transpose(0, 2, 1, 3)
    ).reshape(N_CORES, BLOCKS_PER_CORE * 128, K * 258)

    lid_f = np.full(TOT, -1.0, np.float32)
    lid_f[slot] = (dst & 127).astype(np.float32)
    lid_all = (
        lid_f.reshape(N_CORES, BLOCKS_PER_CORE, K, 128)
        .transpose(0, 3, 1, 2)
        .reshape(N_CORES, 128, BLOCKS_PER_CORE * K)
        .astype(BF16)
    )
    wt = np.ascontiguousarray(W.T)
    bias = np.ascontiguousarray(b.reshape(128, 1))
    ident = np.eye(128, dtype=np.float32)
    iotar = np.ascontiguousarray(
        np.broadcast_to(
            np.arange(128, dtype=np.float32), (128, K, 128)
        ).reshape(128, K * 128)
    ).astype(BF16)

    in_maps = [
        {
            "xe": np.ascontiguousarray(X[c]),
            "lid": np.ascontiguousarray(lid_all[c]),
            "wt": wt,
            "bias": bias,
            "ident": ident,
            "iotar": iotar,
        }
        for c in range(N_CORES)
    ]
    return K, in_maps


def run(edge_data, dst, W, b, trace=False, tmpdir=None):
    from concourse.bass_utils import run_bass_kernel_spmd

    K, in_maps = prepare_inputs(edge_data, dst, W, b)
    nc = _get_module(K)
    res = run_bass_kernel_spmd(
        nc, in_maps, core_ids=list(range(N_CORES)), trace=trace, tmpdir=tmpdir,
    )
    outs = [res.results[c]["out"].T for c in range(N_CORES)]   # [12544, 128] each
    full = np.concatenate(outs, axis=0)[:N_NODES]
    return np.ascontiguousarray(full, dtype=np.float32), res


def kernel(edge_data, dst, W, b):
    out, _ = run(edge_data, dst, W, b, trace=False)
    return out


# revision 20
# speedup vs baseline: 1.1394x; 1.0949x over previous
"""GCN edge-aggregation kernel for 8 Trainium2 NeuronCores.

Math (see nn_GCNEdge): h = relu((segment_sum(edge_data, dst) / max(count,1)) @ W.T + b)

Strategy
--------
Host-side (sharding/layout only — all arithmetic happens on device):
  * Nodes are split contiguously across the 8 cores (12544 = 98 blocks of 128
    nodes per core; 8*12544 = 100352 >= 100000).
  * Each edge is routed to the core/block owning its destination node (CSR-style
    destination binning).  Within a block, edges occupy sequential slots; each
    block is padded to K_CHUNKS*128 slots so the device program is data-independent.
  * Edge features are shipped as a bf16 hi/lo pair (hi = bf16(x),
    lo = bf16(x - hi)) so the on-device f32-accumulated matmuls reconstruct
    ~fp32 precision while streaming at bf16 rates.  A constant-1 column rides
    along for the degree counts.

Device-side (per core, per 128-node block):
  * One-hot matrix of local node ids (DVE is_equal against an iota row),
  * PE matmul-accumulate onehot.T @ [x_hi | 1 | x_lo | 0] into PSUM -> per-node
    feature sums (hi+lo parts) and counts,
  * mean = sums * reciprocal(max(count, 1)),
  * PE transpose, then out = relu(W @ agg.T + b) via a second matmul with the
    (pre-transposed) weight as the stationary operand; output stays transposed
    [out_feat, node] and is un-transposed on the host.

No collectives are needed: output shards are disjoint.
"""

import numpy as np
import ml_dtypes

BF16 = ml_dtypes.bfloat16

N_NODES = 100000
N_EDGES = 1600000
F = 128
N_CORES = 8
BLK = 128                       # nodes per block
BLOCKS_PER_CORE = 98
TOTAL_BLOCKS = N_CORES * BLOCKS_PER_CORE        # 784
NODES_PER_CORE = BLOCKS_PER_CORE * BLK          # 12544
K_CHUNKS = 18                   # 128-edge chunks per block (capacity 2304 edges)

_module_cache = {}


def _build_module(K):
    import concourse.mybir as mybir
    import concourse.tile as tile
    from concourse import bacc

    f32 = mybir.dt.float32
    bf16 = mybir.dt.bfloat16
    RB = K * 128                 # edge slots per block
    SLOTS = BLOCKS_PER_CORE * RB

    nc = bacc.Bacc("TRN2", target_bir_lowering=False, debug=False)
    # xe rows are (block, partition); each row is that partition's K chunks of
    # 258 bf16 values laid contiguously -> 9KB-contiguous DMA descriptors.
    xe = nc.dram_tensor("xe", [BLOCKS_PER_CORE * 128, K * 258], bf16, kind="ExternalInput")
    lid = nc.dram_tensor("lid", [128, BLOCKS_PER_CORE * K], bf16, kind="ExternalInput")
    wt = nc.dram_tensor("wt", [128, 128], f32, kind="ExternalInput")
    bias = nc.dram_tensor("bias", [128, 1], f32, kind="ExternalInput")
    ident = nc.dram_tensor("ident", [128, 128], f32, kind="ExternalInput")
    # iota value pattern tiled K times: iotar[p, c*128 + f] = f
    iotar = nc.dram_tensor("iotar", [128, K * 128], bf16, kind="ExternalInput")
    out = nc.dram_tensor("out", [128, BLOCKS_PER_CORE * 128], f32, kind="ExternalOutput")

    xe_ap = xe.ap()
    out_ap = out.ap()

    with tile.TileContext(nc) as tc:
        with (
            tc.tile_pool(name="const", bufs=1) as cpool,
            tc.tile_pool(name="xp", bufs=6) as xpool,
            tc.tile_pool(name="ohp", bufs=8) as ohpool,
            tc.tile_pool(name="ep", bufs=3) as epool,
            tc.tile_pool(name="psS", bufs=4, space="PSUM") as psS,
            tc.tile_pool(name="psT", bufs=2, space="PSUM") as psT,
            tc.tile_pool(name="psO", bufs=2, space="PSUM") as psO,
        ):
            wt_t = cpool.tile([128, 128], f32)
            nc.sync.dma_start(wt_t[:], wt.ap()[:])
            bias_t = cpool.tile([128, 1], f32)
            nc.sync.dma_start(bias_t[:], bias.ap()[:])
            id_t = cpool.tile([128, 128], f32)
            nc.sync.dma_start(id_t[:], ident.ap()[:])
            iotar_t = cpool.tile([128, K * 128], bf16)
            nc.sync.dma_start(iotar_t[:], iotar.ap()[:])
            lid_t = cpool.tile([128, BLOCKS_PER_CORE * K], bf16)
            nc.sync.dma_start(lid_t[:], lid.ap()[:])

            group_pT = {}

            def emit_matmuls(b, xt, oh):
                ps = psS.tile([128, 258], f32, name=f"ps{b}", tag="ps")
                for c in range(K):
                    nc.tensor.matmul(
                        ps[:],
                        lhsT=oh[:, c * 128:(c + 1) * 128],
                        rhs=xt[:, c * 258:(c + 1) * 258],
                        start=(c == 0),
                        stop=(c == K - 1),
                    )
                return ps

            def emit_epilogue(b, ps):
                # counts live in ps[:,128] (the lo-side count column is all
                # zeros by construction), so no hi+lo add is needed for them.
                cm = epool.tile([128, 1], f32, name=f"cm{b}", tag="cm")
                nc.vector.tensor_scalar_max(cm[:], ps[:, 128:129], 1.0)
                rec = epool.tile([128, 1], f32, name=f"rec{b}", tag="rec")
                nc.vector.reciprocal(rec[:], cm[:])
                # agg = (S_hi + S_lo) / max(count,1), folded as
                # t1 = S_hi*rec (ACT), agg = S_lo*rec + t1 (one fused DVE op)
                t1 = epool.tile([128, 128], f32, name=f"t1{b}", tag="t1")
                nc.scalar.activation(
                    t1[:], ps[:, 0:128],
                    mybir.ActivationFunctionType.Copy, scale=rec[:, 0:1],
                )
                agg = epool.tile([128, 128], f32, name=f"agg{b}", tag="agg")
                nc.vector.scalar_tensor_tensor(
                    out=agg[:],
                    in0=ps[:, 129:257],
                    scalar=rec[:, 0:1],
                    in1=t1[:],
                    op0=mybir.AluOpType.mult,
                    op1=mybir.AluOpType.add,
                )
                j = b % 4
                if j == 0:
                    group_pT["t"] = psT.tile([128, 512], f32, name=f"pT{b}", tag="pT")
                pT = group_pT["t"]
                nc.tensor.transpose(pT[:, j * 128:(j + 1) * 128], agg[:], id_t[:])
                if j == 3 or b == BLOCKS_PER_CORE - 1:
                    g0 = (b // 4) * 4
                    gw = (b + 1 - g0) * 128
                    aggT = epool.tile([128, 512], f32, name=f"aggT{b}", tag="aggT", bufs=2)
                    nc.scalar.copy(aggT[:, 0:gw], pT[:, 0:gw])
                    pO = psO.tile([128, 512], f32, name=f"pO{b}", tag="pO")
                    nc.tensor.matmul(
                        pO[:, 0:gw], lhsT=wt_t[:], rhs=aggT[:, 0:gw],
                        start=True, stop=True,
                    )
                    ot = epool.tile([128, 512], f32, name=f"ot{b}", tag="ot", bufs=2)
                    nc.scalar.activation(
                        ot[:, 0:gw], pO[:, 0:gw],
                        mybir.ActivationFunctionType.Relu,
                        bias=bias_t[:, 0:1], scale=1.0,
                    )
                    nc.sync.dma_start(out_ap[:, g0 * 128:(b + 1) * 128], ot[:, 0:gw])

            # Software-pipelined emission. Engine queues are strict in-order,
            # so any epilogue op that waits on a *just-finished* PSUM
            # accumulation would stall the DVE queue and starve the PE of the
            # next block's one-hot. Emitting each block's epilogue 3 blocks
            # late means its PSUM dependency is long satisfied by the time the
            # DVE queue reaches it, so the queue never blocks.
            EPI_LAG = 3
            pending = {}
            pending_ps = {}
            for b in range(BLOCKS_PER_CORE):
                xt = xpool.tile([128, K * 258], bf16, name=f"xt{b}", tag="xt")
                nc.sync.dma_start(xt[:], xe_ap[b * 128:(b + 1) * 128, :])
                oh = ohpool.tile([128, K * 128], bf16, name=f"oh{b}", tag="oh")
                nc.vector.tensor_tensor(
                    out=oh[:].rearrange("p (c f) -> p c f", c=K),
                    in0=iotar_t[:].rearrange("p (c f) -> p c f", c=K),
                    in1=lid_t[:, b * K:(b + 1) * K].to_broadcast([128, K, 128]),
                    op=mybir.AluOpType.is_equal,
                )
                pending[b] = (xt, oh)
                if b >= 1:
                    pending_ps[b - 1] = emit_matmuls(b - 1, *pending.pop(b - 1))
                if b - 1 - EPI_LAG >= 0 and (b - 1 - EPI_LAG) in pending_ps:
                    bb = b - 1 - EPI_LAG
                    emit_epilogue(bb, pending_ps.pop(bb))
            last = BLOCKS_PER_CORE - 1
            pending_ps[last] = emit_matmuls(last, *pending.pop(last))
            for bb in sorted(pending_ps):
                emit_epilogue(bb, pending_ps.pop(bb))

    nc.compile()
    return nc


def _get_module(K):
    if K not in _module_cache:
        _module_cache[K] = _build_module(K)
    return _module_cache[K]


def prepare_inputs(edge_data, dst, W, b):
    """Host-side sharding: route each edge to the core/block owning dst."""
    edge_data = np.asarray(edge_data, dtype=np.float32)
    dst = np.asarray(dst)
    W = np.asarray(W, dtype=np.float32)
    b = np.asarray(b, dtype=np.float32)
    E = dst.shape[0]

    blk = (dst.astype(np.int64)) >> 7                 # destination block id
    cnt = np.bincount(blk, minlength=TOTAL_BLOCKS)
    K = max(K_CHUNKS, int(np.ceil(cnt.max() / 128)))
    RB = K * 128
    TOT = TOTAL_BLOCKS * RB

    starts = np.zeros(TOTAL_BLOCKS, np.int64)
    np.cumsum(cnt[:-1], out=starts[1:])
    order = np.argsort(blk, kind="stable")
    rank = np.empty(E, np.int64)
    rank[order] = np.arange(E, dtype=np.int64) - np.repeat(starts, cnt)
    slot = blk * RB + rank

    X = np.zeros((TOT, 258), BF16)
    xh = edge_data.astype(BF16)
    X[slot, 0:128] = xh
    X[slot, 128] = BF16(1.0)
    X[slot, 129:257] = (edge_data - xh.astype(np.float32)).astype(BF16)
    # [block, chunk, partition, feat] -> [block, partition, chunk*feat] so each
    # SBUF partition's data is one long contiguous HBM run (big DMA descriptors).
    X = np.ascontiguousarray(
        X.reshape(TOTAL_BLOCKS, K, 128, 258).transpose(0, 2, 1, 3)
    ).reshape(N_CORES, BLOCKS_PER_CORE * 128, K * 258)

    lid_f = np.full(TOT, -1.0, np.float32)
    lid_f[slot] = (dst & 127).astype(np.float32)
    lid_all = (
        lid_f.reshape(N_CORES, BLOCKS_PER_CORE, K, 128)
        .transpose(0, 3, 1, 2)
        .reshape(N_CORES, 128, BLOCKS_PER_CORE * K)
        .astype(BF16)
    )
    wt = np.ascontiguousarray(W.T)
    bias = np.ascontiguousarray(b.reshape(128, 1))
    ident = np.eye(128, dtype=np.float32)
    iotar = np.ascontiguousarray(
        np.broadcast_to(
            np.arange(128, dtype=np.float32), (128, K, 128)
        ).reshape(128, K * 128)
    ).astype(BF16)

    in_maps = [
        {
            "xe": np.ascontiguousarray(X[c]),
            "lid": np.ascontiguousarray(lid_all[c]),
            "wt": wt,
            "bias": bias,
            "ident": ident,
            "iotar": iotar,
        }
        for c in range(N_CORES)
    ]
    return K, in_maps


def run(edge_data, dst, W, b, trace=False, tmpdir=None):
    from concourse.bass_utils import run_bass_kernel_spmd

    K, in_maps = prepare_inputs(edge_data, dst, W, b)
    nc = _get_module(K)
    res = run_bass_kernel_spmd(
        nc, in_maps, core_ids=list(range(N_CORES)), trace=trace, tmpdir=tmpdir,
    )
    outs = [res.results[c]["out"].T for c in range(N_CORES)]   # [12544, 128] each
    full = np.concatenate(outs, axis=0)[:N_NODES]
    return np.ascontiguousarray(full, dtype=np.float32), res


def kernel(edge_data, dst, W, b):
    out, _ = run(edge_data, dst, W, b, trace=False)
    return out
